# revision 29
# baseline (speedup 1.0000x reference)
"""CenterLoss kernel for Trainium2, 8-core SPMD.

Computes (loss, new_centers) for the CenterLoss module:
    loss = mean((features - centers[labels])**2)
    seg_sum[c] = sum of features rows with label c ; counts[c] = multiplicity
    new_centers = where(counts>0, centers + 0.5*(seg_sum/max(counts,1) - centers), centers)

Strategy (data-parallel over batch, 8 NeuronCores):
  - Each core processes a 16384-row shard of features/labels.
  - Per-core segment sums are built with `dma_scatter_add` (SWDGE CCE-add)
    into a DRAM table [10240, 320]: cols 0..255 feature sums, col 256 counts
    (a constant-ones column staged with the features), rest padding (the
    scatter row stride must be a multiple of 256 bytes).
  - Duplicate indices inside one scatter call do NOT accumulate on HW
    (lost-update races in the SDMA read-modify-write), so the host reorders
    each shard into duplicate-free "rounds" (r-th occurrence of each class).
    Consecutive rounds must drain before reusing a row; the table is split
    into 4 class-quarters and each call targets one quarter, so the Tile
    framework's range-based dependency tracking serializes only same-quarter
    (cross-round) calls while descriptor generation for other quarters
    overlaps the in-flight transfers.
  - The host stages features in round-major order with the ones column baked
    in ([gtot, 257] f32), so loads are dense HWDGE DMAs and the Q7 SWDGE
    engine only generates scatter descriptors (the measured bottleneck,
    ~7 ns/descriptor).
  - Loss without gathering centers:
      loss*B*D = sum(f^2) - 2*sum_c seg.centers + sum_c counts*||centers||^2
    sum(f^2) is accumulated by the Scalar engine (Square activation with
    accumulate) while tiles stream through SBUF; the remaining terms are
    computed on each core's class slice after the combine. The final
    combination of 8x3 partial scalars happens on host.
  - Cross-core combine: ReduceScatter(add) over the table; core k keeps
    rows [k*1280, (k+1)*1280) and updates its slice of centers (passed
    pre-sliced per core). Host concatenates the slices and un-maps the
    row space.

Row space: 4 quarters of 2560 rows: quarter q = classes [2500q, 2500q+2500)
at rows [2560q, 2560q+2500), then 60 scratch rows for padding descriptors
(discarded; value races there are harmless).
"""

import numpy as np

import concourse.bacc as bacc
import concourse.mybir as mybir
import concourse.tile as tile
from concourse import bass_utils

P = 128
NCORES = 8
B = 131072
D = 256
C = 10000
NQ = 4  # class quarters
QCLS = C // NQ  # classes per quarter
QROWS = 2560  # rows per quarter (QCLS + 60 scratch)
ROWS = NQ * QROWS  # 10240 table rows
SLICE = ROWS // NCORES  # 1280
SHARD = B // NCORES
ALPHA = 0.5
TW = 320  # table row stride in f32 (multiple of 64 for the scatter)
ES = D + 1  # scatter element: 256 features + ones column
CALL_MAX = 4096  # max indices per scatter call


def _class_rows() -> np.ndarray:
    """row index in the table for each class."""
    c = np.arange(C)
    return QROWS * (c // QCLS) + (c % QCLS)


def _occ_ranks(lab: np.ndarray) -> np.ndarray:
    """occurrence rank of each token within its label."""
    n = lab.shape[0]
    order = np.argsort(lab, kind="stable")
    slab = lab[order]
    is_new = np.empty(n, dtype=bool)
    is_new[0] = True
    is_new[1:] = slab[1:] != slab[:-1]
    grp_start = np.maximum.accumulate(np.where(is_new, np.arange(n), 0))
    occ = np.empty(n, dtype=np.int64)
    occ[order] = np.arange(n) - grp_start
    return occ


def _wrap16(arr: np.ndarray) -> np.ndarray:
    """[G] -> [128, G//16]: token i at (i%16, i//16), replicated 8x."""
    g = arr.shape[0]
    base = arr.reshape(g // 16, 16).T.astype(np.int16)
    return np.tile(base, (8, 1))


def _build_host_data(features: np.ndarray, labels: np.ndarray):
    """Per-core round/quarter grouping, static call grid, staged arrays."""
    # per core, per (round, quarter): token lists
    groups = []  # [core][ (r,q) -> tokens ]
    nrounds = 0
    for c in range(NCORES):
        lab = labels[c * SHARD : (c + 1) * SHARD]
        occ = _occ_ranks(lab)
        q = lab // QCLS
        g = {}
        nr = int(occ.max()) + 1
        nrounds = max(nrounds, nr)
        for r in range(nr):
            in_r = occ == r
            for qq in range(NQ):
                g[(r, qq)] = np.nonzero(in_r & (q == qq))[0]
        groups.append(g)

    # static call grid: for each (round, quarter), size = max over cores,
    # rounded to 16 idxs; split if > CALL_MAX. slot offsets padded to 128.
    # Quarters {0,1} are scheduled before {2,3} so the lower-half
    # ReduceScatter can overlap the upper-half scatters.
    rq_seq = [(r, q) for q in (0, 1) for r in range(nrounds)] + [
        (r, q) for q in (2, 3) for r in range(nrounds)
    ]
    calls = []  # (slot_off, num_idxs, quarter)
    off = 0
    for r, q in rq_seq:
        m = max(len(g.get((r, q), ())) for g in groups)
        if m == 0:
            continue
        left = -(-m // 16) * 16
        while left > 0:
            take = min(left, CALL_MAX)
            calls.append((off, take, q))
            off += -(-take // P) * P  # slot space padded to 128
            left -= take
    gtot = off

    feat_stages = []
    sidx_maps = []
    for c in range(NCORES):
        lab = labels[c * SHARD : (c + 1) * SHARD]
        fsh = features[c * SHARD : (c + 1) * SHARD]
        stage = np.zeros((gtot, ES), dtype=np.float32)
        stage[:, D] = 1.0
        sidx = np.empty(gtot, dtype=np.int64)
        sidx[:] = QCLS + (np.arange(gtot) % 60)  # pad -> scratch rows
        g = groups[c]
        # walk the same grid
        idx_call = 0
        for r, q in rq_seq:
            m = max(len(gg.get((r, q), ())) for gg in groups)
            if m == 0:
                continue
            tok = g.get((r, q), np.zeros(0, np.int64))
            left = -(-m // 16) * 16
            tpos = 0
            while left > 0:
                o, take, qq = calls[idx_call]
                idx_call += 1
                n_here = min(len(tok) - tpos, take)
                n_here = max(n_here, 0)
                if n_here:
                    t = tok[tpos : tpos + n_here]
                    # idx position i in this call lives at SBUF slot
                    # (i%128, i//128); stage it at DRAM row
                    # (i%128)*mb + i//128 so each SBUF partition loads
                    # one contiguous run (coalesced HWDGE descriptors).
                    mslots = -(-take // P) * P
                    mb = mslots // P
                    i = np.arange(n_here)
                    rr = (i % P) * mb + i // P
                    stage[o + rr, :D] = fsh[t]
                    sidx[o : o + n_here] = lab[t] % QCLS
                    tpos += n_here
                left -= take
        feat_stages.append(stage)
        sidx_maps.append(_wrap16(sidx))

    return calls, gtot, feat_stages, sidx_maps


def _build_program(calls, gtot: int):
    nc = bacc.Bacc(
        "TRN2", target_bir_lowering=False, debug=False, num_devices=NCORES
    )
    f32 = mybir.dt.float32
    i16 = mybir.dt.int16

    feat = nc.dram_tensor("feat", [gtot, ES], f32, kind="ExternalInput")
    sidx = nc.dram_tensor("sidx", [P, gtot // 16], i16, kind="ExternalInput")
    cent = nc.dram_tensor("cent", [SLICE, D], f32, kind="ExternalInput")
    out_c = nc.dram_tensor("out_c", [SLICE, D], f32, kind="ExternalOutput")
    out_s = nc.dram_tensor("out_s", [1, 4], f32, kind="ExternalOutput")

    ncalls = len(calls)
    ntiles = SLICE // P
    cmax_b = CALL_MAX // P

    with tile.TileContext(nc) as tc:
        with (
            tc.tile_pool(name="sb", bufs=1) as sb,
            tc.tile_pool(name="sbg", bufs=3) as sbg,
            tc.tile_pool(name="sbc", bufs=2) as sbc,
            tc.tile_pool(name="psum", bufs=1, space="PSUM") as psp,
            tc.tile_pool(name="dram", bufs=1, space="DRAM") as dram,
        ):
            table = dram.tile([ROWS, TW], f32)
            hrows = ROWS // 2
            hslice = SLICE // 2
            rs_a = dram.tile([hslice, TW], f32)
            rs_b = dram.tile([hslice, TW], f32)

            # ---- zero the table ----
            zrows = 2048
            ztile = sb.tile([P, (zrows // P) * TW], f32)
            nc.vector.memset(ztile[:], 0.0)
            for r0 in range(0, ROWS, zrows):
                nc.sync.dma_start(
                    out=table[r0 : r0 + zrows, :].rearrange(
                        "(p n) d -> p (n d)", p=P
                    ),
                    in_=ztile[:],
                )

            sidx_sb = sb.tile([P, gtot // 16], i16)
            nc.sync.dma_start(out=sidx_sb[:], in_=sidx[:])
            a_cols = sb.tile([P, ncalls], f32)

            # ---- phase A: dense load -> square/accum -> scatter ----
            # the last call index touching quarters {0,1}
            k_low_last = max(k for k, c in enumerate(calls) if c[2] < 2)
            for k, (off, m, q) in enumerate(calls):
                mslots = -(-m // P) * P
                mb = mslots // P
                src = sbg.tile([P, cmax_b, ES], f32, tag="src")
                nc.sync.dma_start(
                    out=src[:, :mb, :],
                    in_=feat[off : off + mslots, :].rearrange(
                        "(p n) d -> p n d", p=P
                    ),
                )
                sq = sbc.tile([P, cmax_b, ES], mybir.dt.bfloat16, tag="sq")
                nc.scalar.activation(
                    out=sq[:, :mb, :],
                    in_=src[:, :mb, :],
                    func=mybir.ActivationFunctionType.Square,
                    accum_out=a_cols[:, k : k + 1],
                )
                nc.gpsimd.dma_scatter_add(
                    out_ap=table[q * QROWS : (q + 1) * QROWS, 0:ES],
                    in_ap=src[:, :mb, :],
                    idxs_ap=sidx_sb[:, off // 16 : off // 16 + m // 16],
                    num_idxs=m,
                    num_idxs_reg=m,
                    elem_size=ES,
                    elem_step=TW,
                )
                if k == k_low_last:
                    # lower half done on this core: fire its ReduceScatter so
                    # it overlaps the upper-half scatters
                    nc.gpsimd.collective_compute(
                        "ReduceScatter",
                        mybir.AluOpType.add,
                        replica_groups=[list(range(NCORES))],
                        ins=[table[0:hrows, :].opt()],
                        outs=[rs_a[:].opt()],
                    )

            nc.gpsimd.collective_compute(
                "ReduceScatter",
                mybir.AluOpType.add,
                replica_groups=[list(range(NCORES))],
                ins=[table[hrows:ROWS, :].opt()],
                outs=[rs_b[:].opt()],
            )

            # ---- phase C: center update on this core's slice ----
            b_cols = sb.tile([P, ntiles], f32)
            c_cols = sb.tile([P, ntiles], f32)
            htiles = hslice // P
            for t in range(ntiles):
                rs_src = rs_a if t < htiles else rs_b
                t0 = (t % htiles) * P
                S = sbg.tile([P, TW], f32, tag="S")
                Cc = sbg.tile([P, D], f32, tag="Cc")
                nc.sync.dma_start(out=S[:], in_=rs_src[t0 : t0 + P, :])
                nc.sync.dma_start(out=Cc[:], in_=cent[t * P : (t + 1) * P, :])
                n_ap = S[:, D : D + 1]
                nm = sbc.tile([P, 1], f32, tag="nm")
                nc.vector.tensor_scalar_max(nm[:], n_ap, 1.0)
                inv = sbc.tile([P, 1], f32, tag="inv")
                nc.vector.reciprocal(inv[:], nm[:])
                g = sbc.tile([P, 1], f32, tag="g")
                nc.vector.tensor_scalar(
                    out=g[:],
                    in0=n_ap,
                    scalar1=1.0,
                    scalar2=ALPHA,
                    op0=mybir.AluOpType.min,
                    op1=mybir.AluOpType.mult,
                )
                mean = sbc.tile([P, D], f32, tag="mean")
                nc.vector.tensor_scalar_mul(mean[:], S[:, 0:D], inv[:])
                dlt = sbc.tile([P, D], f32, tag="dlt")
                nc.vector.tensor_tensor(
                    out=dlt[:], in0=mean[:], in1=Cc[:], op=mybir.AluOpType.subtract
                )
                du = sbc.tile([P, D], f32, tag="du")
                nc.vector.tensor_scalar_mul(du[:], dlt[:], g[:])
                oc = sbc.tile([P, D], f32, tag="oc")
                nc.vector.tensor_tensor(
                    out=oc[:], in0=Cc[:], in1=du[:], op=mybir.AluOpType.add
                )
                nc.sync.dma_start(out=out_c[t * P : (t + 1) * P, :], in_=oc[:])
                # loss partials
                prod = sbc.tile([P, D], f32, tag="prod")
                nc.vector.tensor_tensor(
                    out=prod[:], in0=S[:, 0:D], in1=Cc[:], op=mybir.AluOpType.mult
                )
                nc.vector.tensor_reduce(
                    out=b_cols[:, t : t + 1],
                    in_=prod[:],
                    axis=mybir.AxisListType.X,
                    op=mybir.AluOpType.add,
                )
                csq = sbc.tile([P, D], mybir.dt.bfloat16, tag="csq")
                crow = sbc.tile([P, 1], f32, tag="crow")
                nc.scalar.activation(
                    out=csq[:],
                    in_=Cc[:],
                    func=mybir.ActivationFunctionType.Square,
                    accum_out=crow[:],
                )
                nc.vector.tensor_tensor(
                    out=c_cols[:, t : t + 1],
                    in0=crow[:],
                    in1=n_ap,
                    op=mybir.AluOpType.mult,
                )

            # ---- final scalar partition-reduction via PE ----
            stack = sb.tile([P, 4], f32)
            nc.vector.memset(stack[:, 3:4], 0.0)
            nc.vector.tensor_reduce(
                out=stack[:, 0:1], in_=a_cols[:], axis=mybir.AxisListType.X,
                op=mybir.AluOpType.add,
            )
            nc.vector.tensor_reduce(
                out=stack[:, 1:2], in_=b_cols[:], axis=mybir.AxisListType.X,
                op=mybir.AluOpType.add,
            )
            nc.vector.tensor_reduce(
                out=stack[:, 2:3], in_=c_cols[:], axis=mybir.AxisListType.X,
                op=mybir.AluOpType.add,
            )
            ones1 = sb.tile([P, 1], f32)
            nc.vector.memset(ones1[:], 1.0)
            ps = psp.tile([1, 4], f32, space="PSUM")
            nc.tensor.matmul(ps[:], lhsT=ones1[:], rhs=stack[:], start=True, stop=True)
            scal = sb.tile([1, 4], f32)
            nc.vector.tensor_copy(out=scal[:], in_=ps[:])
            nc.sync.dma_start(out=out_s[:], in_=scal[:])

    nc.compile()
    return nc


_PROG_CACHE: dict = {}


def kernel(features: np.ndarray, labels: np.ndarray, centers: np.ndarray):
    import os

    features = np.ascontiguousarray(features, dtype=np.float32)
    labels_i = np.asarray(labels).astype(np.int64)
    centers = np.ascontiguousarray(centers, dtype=np.float32)

    calls, gtot, feat_stages, sidx_maps = _build_host_data(features, labels_i)

    cache_key = (tuple(calls), gtot)
    if cache_key not in _PROG_CACHE:
        _PROG_CACHE[cache_key] = _build_program(calls, gtot)
    nc = _PROG_CACHE[cache_key]

    rows_of_class = _class_rows()
    cent_rows = np.zeros((ROWS, D), dtype=np.float32)
    cent_rows[rows_of_class] = centers

    # core k owns rows [640k, 640k+640) of each table half
    hrows = ROWS // 2
    hslice = SLICE // 2
    in_maps = []
    for c in range(NCORES):
        sl = np.concatenate(
            [
                cent_rows[c * hslice : (c + 1) * hslice],
                cent_rows[hrows + c * hslice : hrows + (c + 1) * hslice],
            ]
        )
        in_maps.append(
            {
                "feat": feat_stages[c],
                "sidx": sidx_maps[c],
                "cent": sl,
            }
        )

    trace = bool(os.environ.get("KERNEL_TRACE"))
    kw = {}
    if trace:
        kw["trace"] = True
        kw["tmpdir"] = os.environ.get("KERNEL_TRACE_DIR") or None
    res = bass_utils.run_bass_kernel_spmd(
        nc, in_maps, core_ids=list(range(NCORES)), **kw
    )
    global LAST_RESULT
    LAST_RESULT = res

    out_rows = np.zeros((ROWS, D), dtype=np.float32)
    for c in range(NCORES):
        oc = res.results[c]["out_c"]
        out_rows[c * hslice : (c + 1) * hslice] = oc[:hslice]
        out_rows[hrows + c * hslice : hrows + (c + 1) * hslice] = oc[hslice:]
    new_centers = out_rows[rows_of_class]
    scal = np.zeros(4, dtype=np.float64)
    for c in range(NCORES):
        scal += res.results[c]["out_s"][0].astype(np.float64)
    a = scal[0] - NCORES * gtot  # remove the ones-column contribution
    loss = (a - 2.0 * scal[1] + scal[2]) / (B * D)
    return np.float32(loss), new_centers


# revision 30
# speedup vs baseline: 1.1798x; 1.1798x over previous
"""CenterLoss kernel for Trainium2, 8-core SPMD.

Computes (loss, new_centers) for the CenterLoss module:
    loss = mean((features - centers[labels])**2)
    seg_sum[c] = sum of features rows with label c ; counts[c] = multiplicity
    new_centers = where(counts>0, centers + 0.5*(seg_sum/max(counts,1) - centers), centers)

Strategy (data-parallel over batch, 8 NeuronCores):
  - Each core processes a 16384-row shard of features/labels.
  - Per-core segment sums are built with `dma_scatter_add` (SWDGE CCE-add)
    into a DRAM table [10240, 320]: cols 0..255 feature sums, col 256 counts
    (a constant-ones column staged with the features), rest padding (the
    scatter row stride must be a multiple of 256 bytes).
  - Duplicate indices inside one scatter call do NOT accumulate on HW
    (lost-update races in the SDMA read-modify-write), so the host reorders
    each shard into duplicate-free "rounds" (r-th occurrence of each class).
    Consecutive rounds must drain before reusing a row; the table is split
    into 4 class-quarters and each call targets one quarter, so the Tile
    framework's range-based dependency tracking serializes only same-quarter
    (cross-round) calls while descriptor generation for other quarters
    overlaps the in-flight transfers.
  - The host stages features in round-major order with the ones column baked
    in ([gtot, 257] f32), so loads are dense HWDGE DMAs and the Q7 SWDGE
    engine only generates scatter descriptors (the measured bottleneck,
    ~7 ns/descriptor).
  - Loss without gathering centers:
      loss*B*D = sum(f^2) - 2*sum_c seg.centers + sum_c counts*||centers||^2
    sum(f^2) is accumulated by the Scalar engine (Square activation with
    accumulate) while tiles stream through SBUF; the remaining terms are
    computed on each core's class slice after the combine. The final
    combination of 8x3 partial scalars happens on host.
  - Cross-core combine: ReduceScatter(add) over the table; core k keeps
    rows [k*1280, (k+1)*1280) and updates its slice of centers (passed
    pre-sliced per core). Host concatenates the slices and un-maps the
    row space.

Row space: 4 quarters of 2560 rows: quarter q = classes [2500q, 2500q+2500)
at rows [2560q, 2560q+2500), then 60 scratch rows for padding descriptors
(discarded; value races there are harmless).
"""

import numpy as np

import concourse.bacc as bacc
import concourse.mybir as mybir
import concourse.tile as tile
from concourse import bass_utils

P = 128
NCORES = 8
B = 131072
D = 256
C = 10000
NQ = 4  # class quarters
QCLS = C // NQ  # classes per quarter
QROWS = 2560  # rows per quarter (QCLS + 60 scratch)
ROWS = NQ * QROWS  # 10240 table rows
SLICE = ROWS // NCORES  # 1280
SHARD = B // NCORES
ALPHA = 0.5
TW = 320  # table row stride in f32 (multiple of 64 for the scatter)
ES = D + 1  # scatter element: 256 features + ones column
CALL_MAX = 4096  # max indices per scatter call


def _class_rows() -> np.ndarray:
    """row index in the table for each class."""
    c = np.arange(C)
    return QROWS * (c // QCLS) + (c % QCLS)


def _occ_ranks(lab: np.ndarray) -> np.ndarray:
    """occurrence rank of each token within its label."""
    n = lab.shape[0]
    order = np.argsort(lab, kind="stable")
    slab = lab[order]
    is_new = np.empty(n, dtype=bool)
    is_new[0] = True
    is_new[1:] = slab[1:] != slab[:-1]
    grp_start = np.maximum.accumulate(np.where(is_new, np.arange(n), 0))
    occ = np.empty(n, dtype=np.int64)
    occ[order] = np.arange(n) - grp_start
    return occ


def _wrap16(arr: np.ndarray) -> np.ndarray:
    """[G] -> [128, G//16]: token i at (i%16, i//16), replicated 8x."""
    g = arr.shape[0]
    base = arr.reshape(g // 16, 16).T.astype(np.int16)
    return np.tile(base, (8, 1))


def _build_host_data(features: np.ndarray, labels: np.ndarray):
    """Per-core round/quarter grouping, static call grid, staged arrays."""
    # per core, per (round, quarter): token lists
    groups = []  # [core][ (r,q) -> tokens ]
    nrounds = 0
    for c in range(NCORES):
        lab = labels[c * SHARD : (c + 1) * SHARD]
        occ = _occ_ranks(lab)
        q = lab // QCLS
        g = {}
        nr = int(occ.max()) + 1
        nrounds = max(nrounds, nr)
        for r in range(nr):
            in_r = occ == r
            for qq in range(NQ):
                g[(r, qq)] = np.nonzero(in_r & (q == qq))[0]
        groups.append(g)

    # static call grid: for each (round, quarter), size = max over cores,
    # rounded to 16 idxs; split if > CALL_MAX. slot offsets padded to 128.
    # Quarters {0,1} are scheduled before {2,3} so the lower-half
    # ReduceScatter can overlap the upper-half scatters.
    rq_seq = [(r, q) for r in range(nrounds) for q in (0, 1)] + [
        (r, q) for r in range(nrounds) for q in (2, 3)
    ]
    calls = []  # (slot_off, num_idxs, quarter)
    off = 0
    for r, q in rq_seq:
        m = max(len(g.get((r, q), ())) for g in groups)
        if m == 0:
            continue
        left = -(-m // 16) * 16
        while left > 0:
            take = min(left, CALL_MAX)
            calls.append((off, take, q))
            off += -(-take // P) * P  # slot space padded to 128
            left -= take
    gtot = off

    feat_stages = []
    sidx_maps = []
    for c in range(NCORES):
        lab = labels[c * SHARD : (c + 1) * SHARD]
        fsh = features[c * SHARD : (c + 1) * SHARD]
        stage = np.zeros((gtot, ES), dtype=np.float32)
        stage[:, D] = 1.0
        sidx = np.empty(gtot, dtype=np.int64)
        sidx[:] = QCLS + (np.arange(gtot) % 60)  # pad -> scratch rows
        g = groups[c]
        # walk the same grid
        idx_call = 0
        for r, q in rq_seq:
            m = max(len(gg.get((r, q), ())) for gg in groups)
            if m == 0:
                continue
            tok = g.get((r, q), np.zeros(0, np.int64))
            left = -(-m // 16) * 16
            tpos = 0
            while left > 0:
                o, take, qq = calls[idx_call]
                idx_call += 1
                n_here = min(len(tok) - tpos, take)
                n_here = max(n_here, 0)
                if n_here:
                    t = tok[tpos : tpos + n_here]
                    # idx position i in this call lives at SBUF slot
                    # (i%128, i//128); stage it at DRAM row
                    # (i%128)*mb + i//128 so each SBUF partition loads
                    # one contiguous run (coalesced HWDGE descriptors).
                    mslots = -(-take // P) * P
                    mb = mslots // P
                    i = np.arange(n_here)
                    rr = (i % P) * mb + i // P
                    stage[o + rr, :D] = fsh[t]
                    sidx[o : o + n_here] = lab[t] % QCLS
                    tpos += n_here
                left -= take
        feat_stages.append(stage)
        sidx_maps.append(_wrap16(sidx))

    return calls, gtot, feat_stages, sidx_maps


def _build_program(calls, gtot: int):
    nc = bacc.Bacc(
        "TRN2", target_bir_lowering=False, debug=False, num_devices=NCORES
    )
    f32 = mybir.dt.float32
    i16 = mybir.dt.int16

    feat = nc.dram_tensor("feat", [gtot, ES], f32, kind="ExternalInput")
    sidx = nc.dram_tensor("sidx", [P, gtot // 16], i16, kind="ExternalInput")
    cent = nc.dram_tensor("cent", [SLICE, D], f32, kind="ExternalInput")
    out_c = nc.dram_tensor("out_c", [SLICE, D], f32, kind="ExternalOutput")
    out_s = nc.dram_tensor("out_s", [1, 4], f32, kind="ExternalOutput")

    ncalls = len(calls)
    ntiles = SLICE // P
    cmax_b = CALL_MAX // P

    with tile.TileContext(nc) as tc:
        with (
            tc.tile_pool(name="sb", bufs=1) as sb,
            tc.tile_pool(name="sbg", bufs=3) as sbg,
            tc.tile_pool(name="sbc", bufs=2) as sbc,
            tc.tile_pool(name="psum", bufs=1, space="PSUM") as psp,
            tc.tile_pool(name="dram", bufs=1, space="DRAM") as dram,
        ):
            table = dram.tile([ROWS, TW], f32)
            hrows = ROWS // 2
            hslice = SLICE // 2
            rs_a = dram.tile([hslice, TW], f32)
            rs_b = dram.tile([hslice, TW], f32)

            # ---- zero the table ----
            zrows = 2048
            ztile = sb.tile([P, (zrows // P) * TW], f32)
            nc.vector.memset(ztile[:], 0.0)
            for r0 in range(0, ROWS, zrows):
                nc.sync.dma_start(
                    out=table[r0 : r0 + zrows, :].rearrange(
                        "(p n) d -> p (n d)", p=P
                    ),
                    in_=ztile[:],
                )

            sidx_sb = sb.tile([P, gtot // 16], i16)
            nc.sync.dma_start(out=sidx_sb[:], in_=sidx[:])
            a_cols = sb.tile([P, ncalls], f32)

            # ---- phase A: dense load -> square/accum -> scatter ----
            # the last call index touching quarters {0,1}
            k_low_last = max(k for k, c in enumerate(calls) if c[2] < 2)
            for k, (off, m, q) in enumerate(calls):
                mslots = -(-m // P) * P
                mb = mslots // P
                src = sbg.tile([P, cmax_b, ES], f32, tag="src")
                nc.sync.dma_start(
                    out=src[:, :mb, :],
                    in_=feat[off : off + mslots, :].rearrange(
                        "(p n) d -> p n d", p=P
                    ),
                )
                sq = sbc.tile([P, cmax_b, ES], mybir.dt.bfloat16, tag="sq")
                nc.scalar.activation(
                    out=sq[:, :mb, :],
                    in_=src[:, :mb, :],
                    func=mybir.ActivationFunctionType.Square,
                    accum_out=a_cols[:, k : k + 1],
                )
                nc.gpsimd.dma_scatter_add(
                    out_ap=table[q * QROWS : (q + 1) * QROWS, 0:ES],
                    in_ap=src[:, :mb, :],
                    idxs_ap=sidx_sb[:, off // 16 : off // 16 + m // 16],
                    num_idxs=m,
                    num_idxs_reg=m,
                    elem_size=ES,
                    elem_step=TW,
                )
                if k == k_low_last:
                    # lower half done on this core: fire its ReduceScatter so
                    # it overlaps the upper-half scatters
                    nc.gpsimd.collective_compute(
                        "ReduceScatter",
                        mybir.AluOpType.add,
                        replica_groups=[list(range(NCORES))],
                        ins=[table[0:hrows, :].opt()],
                        outs=[rs_a[:].opt()],
                    )

            nc.gpsimd.collective_compute(
                "ReduceScatter",
                mybir.AluOpType.add,
                replica_groups=[list(range(NCORES))],
                ins=[table[hrows:ROWS, :].opt()],
                outs=[rs_b[:].opt()],
            )

            # ---- phase C: center update on this core's slice ----
            b_cols = sb.tile([P, ntiles], f32)
            c_cols = sb.tile([P, ntiles], f32)
            htiles = hslice // P
            for t in range(ntiles):
                rs_src = rs_a if t < htiles else rs_b
                t0 = (t % htiles) * P
                S = sbg.tile([P, TW], f32, tag="S")
                Cc = sbg.tile([P, D], f32, tag="Cc")
                nc.sync.dma_start(out=S[:], in_=rs_src[t0 : t0 + P, :])
                nc.sync.dma_start(out=Cc[:], in_=cent[t * P : (t + 1) * P, :])
                n_ap = S[:, D : D + 1]
                nm = sbc.tile([P, 1], f32, tag="nm")
                nc.vector.tensor_scalar_max(nm[:], n_ap, 1.0)
                inv = sbc.tile([P, 1], f32, tag="inv")
                nc.vector.reciprocal(inv[:], nm[:])
                g = sbc.tile([P, 1], f32, tag="g")
                nc.vector.tensor_scalar(
                    out=g[:],
                    in0=n_ap,
                    scalar1=1.0,
                    scalar2=ALPHA,
                    op0=mybir.AluOpType.min,
                    op1=mybir.AluOpType.mult,
                )
                mean = sbc.tile([P, D], f32, tag="mean")
                nc.vector.tensor_scalar_mul(mean[:], S[:, 0:D], inv[:])
                dlt = sbc.tile([P, D], f32, tag="dlt")
                nc.vector.tensor_tensor(
                    out=dlt[:], in0=mean[:], in1=Cc[:], op=mybir.AluOpType.subtract
                )
                du = sbc.tile([P, D], f32, tag="du")
                nc.vector.tensor_scalar_mul(du[:], dlt[:], g[:])
                oc = sbc.tile([P, D], f32, tag="oc")
                nc.vector.tensor_tensor(
                    out=oc[:], in0=Cc[:], in1=du[:], op=mybir.AluOpType.add
                )
                nc.sync.dma_start(out=out_c[t * P : (t + 1) * P, :], in_=oc[:])
                # loss partials
                prod = sbc.tile([P, D], f32, tag="prod")
                nc.vector.tensor_tensor(
                    out=prod[:], in0=S[:, 0:D], in1=Cc[:], op=mybir.AluOpType.mult
                )
                nc.vector.tensor_reduce(
                    out=b_cols[:, t : t + 1],
                    in_=prod[:],
                    axis=mybir.AxisListType.X,
                    op=mybir.AluOpType.add,
                )
                csq = sbc.tile([P, D], mybir.dt.bfloat16, tag="csq")
                crow = sbc.tile([P, 1], f32, tag="crow")
                nc.scalar.activation(
                    out=csq[:],
                    in_=Cc[:],
                    func=mybir.ActivationFunctionType.Square,
                    accum_out=crow[:],
                )
                nc.vector.tensor_tensor(
                    out=c_cols[:, t : t + 1],
                    in0=crow[:],
                    in1=n_ap,
                    op=mybir.AluOpType.mult,
                )

            # ---- final scalar partition-reduction via PE ----
            stack = sb.tile([P, 4], f32)
            nc.vector.memset(stack[:, 3:4], 0.0)
            nc.vector.tensor_reduce(
                out=stack[:, 0:1], in_=a_cols[:], axis=mybir.AxisListType.X,
                op=mybir.AluOpType.add,
            )
            nc.vector.tensor_reduce(
                out=stack[:, 1:2], in_=b_cols[:], axis=mybir.AxisListType.X,
                op=mybir.AluOpType.add,
            )
            nc.vector.tensor_reduce(
                out=stack[:, 2:3], in_=c_cols[:], axis=mybir.AxisListType.X,
                op=mybir.AluOpType.add,
            )
            ones1 = sb.tile([P, 1], f32)
            nc.vector.memset(ones1[:], 1.0)
            ps = psp.tile([1, 4], f32, space="PSUM")
            nc.tensor.matmul(ps[:], lhsT=ones1[:], rhs=stack[:], start=True, stop=True)
            scal = sb.tile([1, 4], f32)
            nc.vector.tensor_copy(out=scal[:], in_=ps[:])
            nc.sync.dma_start(out=out_s[:], in_=scal[:])

    nc.compile()
    return nc


_PROG_CACHE: dict = {}


def kernel(features: np.ndarray, labels: np.ndarray, centers: np.ndarray):
    import os

    features = np.ascontiguousarray(features, dtype=np.float32)
    labels_i = np.asarray(labels).astype(np.int64)
    centers = np.ascontiguousarray(centers, dtype=np.float32)

    calls, gtot, feat_stages, sidx_maps = _build_host_data(features, labels_i)

    cache_key = (tuple(calls), gtot)
    if cache_key not in _PROG_CACHE:
        _PROG_CACHE[cache_key] = _build_program(calls, gtot)
    nc = _PROG_CACHE[cache_key]

    rows_of_class = _class_rows()
    cent_rows = np.zeros((ROWS, D), dtype=np.float32)
    cent_rows[rows_of_class] = centers

    # core k owns rows [640k, 640k+640) of each table half
    hrows = ROWS // 2
    hslice = SLICE // 2
    in_maps = []
    for c in range(NCORES):
        sl = np.concatenate(
            [
                cent_rows[c * hslice : (c + 1) * hslice],
                cent_rows[hrows + c * hslice : hrows + (c + 1) * hslice],
            ]
        )
        in_maps.append(
            {
                "feat": feat_stages[c],
                "sidx": sidx_maps[c],
                "cent": sl,
            }
        )

    trace = bool(os.environ.get("KERNEL_TRACE"))
    kw = {}
    if trace:
        kw["trace"] = True
        kw["tmpdir"] = os.environ.get("KERNEL_TRACE_DIR") or None
    res = bass_utils.run_bass_kernel_spmd(
        nc, in_maps, core_ids=list(range(NCORES)), **kw
    )
    global LAST_RESULT
    LAST_RESULT = res

    out_rows = np.zeros((ROWS, D), dtype=np.float32)
    for c in range(NCORES):
        oc = res.results[c]["out_c"]
        out_rows[c * hslice : (c + 1) * hslice] = oc[:hslice]
        out_rows[hrows + c * hslice : hrows + (c + 1) * hslice] = oc[hslice:]
    new_centers = out_rows[rows_of_class]
    scal = np.zeros(4, dtype=np.float64)
    for c in range(NCORES):
        scal += res.results[c]["out_s"][0].astype(np.float64)
    a = scal[0] - NCORES * gtot  # remove the ones-column contribution
    loss = (a - 2.0 * scal[1] + scal[2]) / (B * D)
    return np.float32(loss), new_centers
